# revision 35
# baseline (speedup 1.0000x reference)
"""Trainium2 Bass kernel for nn_CustomGPT2Attention (B=2, S=2048, D=1024, H=16).

Sharding: Megatron-style head-parallel over 8 cores (2 heads/core).
Each core computes QKV projection for its 2 heads, RoPE, causal
attention, and a row-parallel c_proj partial [D, T]; the host sums the
8 partials and adds b_proj.

v2 changes vs the fp32r baseline (295.7us):
  * all matmul operands bf16 (fp32 streams ~3x slower per column and
    disables FWL on LDWEIGHTS); PSUM accumulation stays fp32
  * softmax 1/den via nc.vector.reciprocal (DVE) instead of ACT ln+exp,
    which thrashed ACT tables (17 x ~1.3us ACT_TABLE_LOAD per run)
  * diag-block exp issued as one 3D-AP ACTIVATE (fewer ACT fixed costs)
  * rope rotate-half staged with 2 partition-permuted DMAs (was 4)
  * outT + DMA in bf16 (halves output HBM traffic; host sums in f64)
  * proj PSUM->SBUF copies split across DVE and ACT to balance engines

Per-core data layout (features on partitions, "transposed"):
  xT      [D, T]    full hidden, transposed  (T = B*S tokens)
  qT/kT   [128, T]  partitions = (2 heads x 64 hd)
  scores  S^T tile [j=128, i<=512] so softmax'd probs feed the
          attn@V matmul directly as the moving operand
  V       [t, hd] via PE transpose, with a ones-column appended so the
          softmax denominator rides the attn@V matmul (M=65)
  1/den   via DVE reciprocal on the merged [1,1024] denominator row,
          then a PE ones-matmul partition-broadcast
  out     partial^T [D, T], host sums across cores

The attention stream is ACT(exp)-paced; QKV(b1) and c_proj chunks are
split into small units and interleaved one-per-f into the attention
emission so the PE FIFO stays dense (HAM stays warm) without starving
the exp stream.
"""

import numpy as np
from collections import deque
from contextlib import ExitStack

import concourse.bass as bass
from concourse import bacc
import concourse.mybir as mybir
import concourse.tile as tile
from concourse.bass import ts, ds
from concourse.bass_utils import run_bass_kernel_spmd
from concourse.masks import make_identity, make_upper_triangular

F32 = mybir.dt.float32
EXP = mybir.ActivationFunctionType.Exp
LN = mybir.ActivationFunctionType.Ln

# Both Ln and Exp live in the `natural_log_exp_and_others` ACT table set,
# but the table-load placement pass resolves each function to the FIRST
# set containing it (exp -> exp_and_others, ln -> natural_log), which
# thrashes ACT_TABLE_LOADs (~2.7us each) every attention chunk. Restrict
# the advertised contents of those two sets (without disturbing set IDs)
# so both functions resolve to the shared set and one load covers all.
_ACT_TABLES_PATCHED = False


def _patch_act_tables():
    global _ACT_TABLES_PATCHED
    if _ACT_TABLES_PATCHED:
        return
    import concourse.hw_specs as hw_specs

    orig = hw_specs.get_activation_tables.__wrapped__

    @__import__("functools").cache
    def patched(module_arch):
        tables = {k: set(v) for k, v in orig(module_arch).items()}
        if "natural_log_exp_and_others" in tables:
            tables.get("exp_and_others", set()).discard(EXP)
            tables.get("natural_log", set()).discard(LN)
        return tables

    hw_specs.get_activation_tables = patched
    bacc.get_activation_tables = patched  # bacc holds a direct reference
    _ACT_TABLES_PATCHED = True

B, S, D = 2, 2048, 1024
H, HD = 16, 64
NCORES = 8
HPC = H // NCORES            # heads per core = 2
FL = HPC * HD                # local features = 128
THETA = 10000.0
TC = 512                     # token chunk (qkv / proj)
SC = 512                     # query chunk (attention)
JB = 128                     # key block
SCALE = 1.0 / 8.0            # 1/sqrt(HD)

MM_DT = mybir.dt.bfloat16    # matmul operand dtype


def build_nc(S_=S):
    _patch_act_tables()
    T = B * S_
    NCC = S_ // SC
    NTCB = S_ // TC
    NJT = T // JB
    NDT = D // 128

    nc = bacc.Bacc("TRN2", target_bir_lowering=False)
    xT = nc.declare_dram_parameter("xT", [D, T], MM_DT, isOutput=False)
    wqkv = nc.declare_dram_parameter("wqkv", [128, 3 * NDT * 128], MM_DT, isOutput=False)
    bqkv = nc.declare_dram_parameter("bqkv", [FL, 3], F32, isOutput=False)
    wproj = nc.declare_dram_parameter("wproj", [FL, D], MM_DT, isOutput=False)
    cos2 = nc.declare_dram_parameter("cos2", [FL, S_], MM_DT, isOutput=False)
    sin2s = nc.declare_dram_parameter("sin2s", [FL, S_], MM_DT, isOutput=False)
    outT = nc.declare_dram_parameter("outT", [D, T], MM_DT, isOutput=True)

    with tile.TileContext(nc) as tc:
        with ExitStack() as ctx:
            cpool = ctx.enter_context(tc.tile_pool(name="consts", bufs=1))
            big = ctx.enter_context(tc.tile_pool(name="big", bufs=1))
            xtp = ctx.enter_context(tc.tile_pool(name="xt", bufs=2))
            rpp = ctx.enter_context(tc.tile_pool(name="rope", bufs=2))
            ppp = ctx.enter_context(tc.tile_pool(name="pp", bufs=3))
            smp = ctx.enter_context(tc.tile_pool(name="small", bufs=2))
            stg = ctx.enter_context(tc.tile_pool(name="stg", bufs=3))
            mmps = ctx.enter_context(tc.tile_pool(name="mmps", bufs=2, space="PSUM"))
            scps = ctx.enter_context(tc.tile_pool(name="scps", bufs=2, space="PSUM"))
            ops = ctx.enter_context(tc.tile_pool(name="ops", bufs=1, space="PSUM"))

            # ---- weights first on the SP ring (QKV needs them first) ----
            # wqkv is pre-swizzled on the host to the exact SBUF layout and
            # split per-ft so the first matmul only waits on 1/3 of it
            wq_sb = [
                cpool.tile([128, NDT * 128], MM_DT, name=f"wq_sb{ft}")
                for ft in range(3)
            ]
            for ft in range(3):
                nc.sync.dma_start(wq_sb[ft][:], wqkv[:, ds(ft * NDT * 128, NDT * 128)])
            # ---- prefetch x chunk 0 on the ACT ring before the tables,
            # split in half so the first QKV matmuls start even earlier ----
            xT_r = xT.rearrange("(dk p) t -> p dk t", p=128)
            HK = NDT // 2
            xt0 = xtp.tile([128, NDT, TC], MM_DT, name="xt0")
            nc.scalar.dma_start(xt0[:, ds(0, HK), :], xT_r[:, ds(0, HK), ds(0, TC)])
            nc.scalar.dma_start(xt0[:, ds(HK, HK), :], xT_r[:, ds(HK, HK), ds(0, TC)])
            # ---- other constants on the ACT ring (parallel HWDGE ring) ----
            cos_sb = cpool.tile([128, S_], MM_DT)
            nc.scalar.dma_start(cos_sb[:], cos2[:, :])
            sin_sb = cpool.tile([128, S_], MM_DT)
            nc.scalar.dma_start(sin_sb[:], sin2s[:, :])
            bq_sb = cpool.tile([128, 3], F32)
            nc.scalar.dma_start(bq_sb[:], bqkv[:, :])
            wp_sb = cpool.tile([128, D], MM_DT)
            nc.scalar.dma_start(wp_sb[:], wproj[:, :])
            ident = cpool.tile([128, 128], MM_DT)
            make_identity(nc, ident[:])
            diagm = cpool.tile([128, 128], MM_DT)
            make_upper_triangular(nc, diagm[:], val=1.0, diag=True)
            ones64 = cpool.tile([1, 64], MM_DT)
            nc.vector.memset(ones64[:], 1.0)

            # ---- persistent activations ----
            q_sb = big.tile([128, T], MM_DT)
            k_sb = big.tile([128, T], MM_DT)
            vT_sb = big.tile([128, T], MM_DT)
            v_sb = big.tile([128, NJT * 130], MM_DT)  # [h0|1|h1|1] per block
            oT_sb = big.tile([128, T], MM_DT)
            nc.gpsimd.memset(v_sb[:], 1.0)

            # ------------------------------------------------------ units --
            def u_qkv_ft(b, cb, ft, xt):
                c = b * NTCB + cb
                t0 = c * TC
                if ft == 0 and c != 0:  # chunk 0 was prefetched above
                    nc.sync.dma_start(xt[:], xT_r[:, :, ds(t0, TC)])
                ps = mmps.tile([128, TC], F32, tag="mmps", name="ps")
                for dk in range(NDT):
                    nc.tensor.matmul(
                        ps[:],
                        wq_sb[ft][:, ts(dk, 128)],
                        xt[:, dk, :],
                        start=(dk == 0),
                        stop=(dk == NDT - 1),
                    )
                dst = (q_sb, k_sb, vT_sb)[ft]
                nc.vector.tensor_scalar_add(
                    dst[:, ds(t0, TC)], ps[:], bq_sb[:, ds(ft, 1)]
                )
                if ft >= 1:
                    # rope on q (ft==1) / k (ft==2) of this chunk
                    xsb = (q_sb, k_sb)[ft - 1]
                    s0 = t0 - b * S_
                    rot = rpp.tile([128, TC], MM_DT, tag="rot", name="rot")
                    for (po, pi) in ((0, 32), (32, 0), (64, 96), (96, 64)):
                        nc.gpsimd.dma_start(
                            rot[ds(po, 32), :], xsb[ds(pi, 32), ds(t0, TC)]
                        )
                    tmp = rpp.tile([128, TC], MM_DT, tag="tmp", name="tmp")
                    nc.vector.tensor_mul(
                        tmp[:], xsb[:, ds(t0, TC)], cos_sb[:, ds(s0, TC)]
                    )
                    nc.vector.tensor_mul(rot[:], rot[:], sin_sb[:, ds(s0, TC)])
                    nc.vector.tensor_add(xsb[:, ds(t0, TC)], tmp[:], rot[:])

            def u_vtrans(b, cb, jj):
                c = b * NTCB + cb
                jt = c * (TC // JB) + jj
                tp = mmps.tile([128, 128], MM_DT, tag="mmps", name="tp")
                nc.tensor.transpose(tp[:], vT_sb[:, ts(jt, JB)], ident[:])
                nc.vector.tensor_copy(
                    v_sb[:, ds(130 * jt, 130)].rearrange("p (g n) -> p g n", g=2)[
                        :, :, ds(0, 64)
                    ],
                    tp[:].rearrange("p (g n) -> p g n", g=2),
                )

            def u_proj(b, cc, dt):
                c = b * NTCB + cc
                pj = mmps.tile([128, TC], F32, tag="mmps", name="pj")
                nc.tensor.matmul(
                    pj[:], wp_sb[:, ts(dt, 128)], oT_sb[:, ts(c, TC)],
                    start=True, stop=True,
                )
                so = stg.tile([128, TC], MM_DT, tag="stg", name="so")
                if dt % 2 == 0:
                    nc.vector.tensor_copy(so[:], pj[:])
                else:
                    nc.scalar.copy(so[:], pj[:])
                nc.sync.dma_start(outT[ds(dt * 128, 128), ds(c * TC, TC)], so[:])

            UPC = 3 + TC // JB  # units per qkv chunk

            def qkv_units(b, cb):
                xt = xt0 if (b, cb) == (0, 0) else xtp.tile(
                    [128, NDT, TC], MM_DT, name="xt"
                )
                for ft in range(3):
                    yield (lambda b=b, cb=cb, ft=ft, xt=xt: u_qkv_ft(b, cb, ft, xt))
                for jj in range(TC // JB):
                    yield (lambda b=b, cb=cb, jj=jj: u_vtrans(b, cb, jj))

            fill_qkv = deque()
            fill_proj = deque()
            # pace filler consumption so the supply lasts through the final
            # attention chunk (an empty filler queue leaves the PE idling on
            # the exp stream, which re-throttles HAM)
            slots_left = [B * sum(4 * cc + 4 + 1 for cc in range(NCC))]

            def pop_filler():
                supply = len(fill_qkv) + len(fill_proj)
                k = min(2, max(1, -(-supply // max(slots_left[0], 1))))
                slots_left[0] -= 1
                for _ in range(k):
                    if fill_qkv:
                        fill_qkv.popleft()()
                    elif fill_proj:
                        fill_proj.popleft()()

            pending_finish = [None]

            def attn_finish(b, cc, oph2):
                # normalize: 1/d = exp(-ln d) on ACT, then PE broadcast.
                # Deferred into the NEXT chunk's attention body so the PE
                # queue never parks on the rc dependency at a boundary.
                i0 = b * S_ + cc * SC
                lnd = smp.tile([1, 2 * SC], F32, tag="lnd", name="lnd")
                nc.scalar.activation(lnd[:], oph2[ds(64, 1), :], LN)
                rc = smp.tile([1, 2 * SC], MM_DT, tag="rc", name="rc")
                nc.scalar.activation(rc[:], lnd[:], EXP, scale=-1.0)
                bcs = smp.tile([64, 2 * SC], MM_DT, tag="bcs", name="bcs")
                for h in range(2):
                    bcp = mmps.tile([64, SC], F32, tag="mmps", name="bcp")
                    nc.tensor.matmul(
                        bcp[:], ones64[:], rc[:, ds(SC * h, SC)],
                        start=True, stop=True,
                    )
                    nc.vector.tensor_copy(bcs[:, ds(SC * h, SC)], bcp[:])
                for h in range(2):
                    nc.vector.tensor_mul(
                        oT_sb[ds(64 * h, 64), ds(i0, SC)],
                        oph2[ds(0, 64), ds(SC * h, SC)],
                        bcs[:, ds(SC * h, SC)],
                    )
                for dt in range(NDT):
                    fill_proj.append(lambda b=b, cc=cc, dt=dt: u_proj(b, cc, dt))

            def emit_attn(b, cc):
                oph2 = ops.tile([65, 2 * SC], F32, tag="ops", name="oph2")
                nf = 4 * cc + 4

                def mk_scores(f):
                    ist = max(SC * cc, JB * f)
                    off = ist - SC * cc
                    N = SC - off
                    scp = scps.tile([128, 2 * SC], F32, tag="scps", name="scp")
                    for h in range(2):
                        nc.tensor.matmul(
                            scp[:, ds(SC * h + off, N)],
                            k_sb[ds(64 * h, 64), ds(b * S_ + JB * f, JB)],
                            q_sb[ds(64 * h, 64), ds(b * S_ + ist, N)],
                            start=True,
                            stop=True,
                        )
                    pp = ppp.tile([128, 2 * SC], MM_DT, tag="pp", name="pp")
                    if off == 0:
                        nc.scalar.activation(pp[:], scp[:], EXP, scale=SCALE)
                    else:
                        nc.scalar.activation(
                            pp[:].rearrange("p (g n) -> p g n", g=2)[
                                :, :, ds(off, N)
                            ],
                            scp[:].rearrange("p (g n) -> p g n", g=2)[
                                :, :, ds(off, N)
                            ],
                            EXP,
                            scale=SCALE,
                        )
                    if f >= 4 * cc:  # diagonal block: zero j > i
                        pp3 = pp[:].rearrange("p (g n) -> p g n", g=2)[
                            :, :, ds(off, JB)
                        ]
                        nc.vector.tensor_mul(
                            pp3, pp3, diagm[:].unsqueeze(1).to_broadcast((128, 2, JB))
                        )
                    return pp, off, N

                def mk_attnv(f, pp, off, N):
                    jt = b * (S_ // JB) + f
                    for h in range(2):
                        nc.tensor.matmul(
                            oph2[:, ds(SC * h + off, N)],
                            v_sb[:, ds(130 * jt + 65 * h, 65)],
                            pp[:, ds(SC * h + off, N)],
                            start=(f == 0),
                            stop=(f == nf - 1),
                        )

                # software-pipelined: scores run one f ahead of attn@V so the
                # PE FIFO never parks on an exp-dependent matmul; the prior
                # chunk's finish work slots in behind the first scores
                prev = mk_scores(0)
                start_f = 1
                if pending_finish[0] is not None:
                    # prior chunk's finish: emit it after two scores steps
                    # and two fillers so its rc-dependent broadcast never
                    # parks the PE, but before this chunk's first attn@V
                    # (which recycles the single-buffered oph2 PSUM tile)
                    cur = mk_scores(1)
                    pop_filler()
                    pop_filler()
                    pending_finish[0]()
                    pending_finish[0] = None
                    mk_attnv(0, *prev)
                    prev = cur
                    start_f = 2
                for f in range(start_f, nf):
                    cur = mk_scores(f)
                    mk_attnv(f - 1, *prev)
                    prev = cur
                    pop_filler()
                mk_attnv(nf - 1, *prev)
                pop_filler()
                pop_filler()
                pending_finish[0] = lambda: attn_finish(b, cc, oph2)

            # ---------------------------------------------------- program --
            # chunk-interleaved: attention for (b, cc) starts as soon as the
            # qkv chunks it reads are done; remaining qkv/proj work fills
            # attention gaps so the PE stays dense (HAM stays warm)
            total_qkv = (2 * NTCB - 1) * UPC
            for u in qkv_units(0, 0):
                u()
            for bb in range(B):
                for cb in range(NTCB):
                    if (bb, cb) != (0, 0):
                        fill_qkv.extend(qkv_units(bb, cb))
            assert len(fill_qkv) == total_qkv

            def chunks_ready():
                return 1 + (total_qkv - len(fill_qkv)) // UPC

            for bb in range(B):
                for cc in range(NCC):
                    g = bb * NCC + cc  # needs qkv chunks 0..g
                    while fill_qkv and chunks_ready() < g + 1:
                        fill_qkv.popleft()()
                    emit_attn(bb, cc)
            if pending_finish[0] is not None:
                pending_finish[0]()
                pending_finish[0] = None
            while fill_qkv:
                fill_qkv.popleft()()
            while fill_proj:
                fill_proj.popleft()()

    nc.finalize()
    return nc


# ---------------------------------------------------------------------------
# host side
# ---------------------------------------------------------------------------

def rope_tables(S_=S):
    hd_half = HD // 2
    inv = (
        np.float32(1.0)
        / np.float32(THETA) ** (np.arange(0, HD, 2, dtype=np.float32) / np.float32(HD))
    ).astype(np.float32)
    t = np.arange(S_, dtype=np.float32)
    freqs = np.outer(t, inv).astype(np.float32)
    emb = np.concatenate([freqs, freqs], axis=1)
    cos = np.cos(emb).astype(np.float32)
    sin = np.sin(emb).astype(np.float32)
    sign = np.where(np.arange(HD) < hd_half, np.float32(-1.0), np.float32(1.0))
    cos2 = np.tile(cos.T, (HPC, 1)).astype(np.float32)
    sin2s = np.tile((sin * sign[None, :]).T, (HPC, 1)).astype(np.float32)
    return np.ascontiguousarray(cos2), np.ascontiguousarray(sin2s)


def make_in_maps(hidden_states, W_qkv, b_qkv, W_proj, S_=S):
    T = B * S_
    mmnp = mybir.dt.np(MM_DT)
    x = np.asarray(hidden_states, dtype=np.float32).reshape(T, D)
    xT = np.ascontiguousarray(x.T).astype(mmnp)
    cos2, sin2s = rope_tables(S_)
    cos2 = cos2.astype(mmnp)
    sin2s = sin2s.astype(mmnp)
    maps = []
    NDT = D // 128
    for i in range(NCORES):
        cs = slice(FL * i, FL * (i + 1))
        # pre-swizzle to the SBUF layout [p, (ft dk c)] so the device DMA
        # is one dense per-partition transfer
        w3 = np.stack([W_qkv[:, k * D:][:, cs] for k in range(3)], axis=0)
        w3 = w3.reshape(3, NDT, 128, FL).transpose(2, 0, 1, 3)
        wq = np.ascontiguousarray(w3.reshape(128, 3 * NDT * FL)).astype(mmnp)
        bq = np.ascontiguousarray(
            np.stack([b_qkv[k * D:][cs] for k in range(3)], axis=1)
        ).astype(np.float32)
        wp = np.ascontiguousarray(W_proj[cs, :]).astype(mmnp)
        maps.append(dict(xT=xT, wqkv=wq, bqkv=bq, wproj=wp, cos2=cos2, sin2s=sin2s))
    return maps


_NC_CACHE = {}


def get_nc(S_=S):
    if S_ not in _NC_CACHE:
        _NC_CACHE[S_] = build_nc(S_)
    return _NC_CACHE[S_]


def gather(results, b_proj, S_=S):
    acc = np.zeros((D, B * S_), dtype=np.float64)
    for r in results:
        acc += np.asarray(r["outT"], dtype=np.float64)
    out = acc.T + np.asarray(b_proj, dtype=np.float64)[None, :]
    return out.astype(np.float32).reshape(B, S_, D)


def kernel(hidden_states, W_qkv, b_qkv, W_proj, b_proj):
    nc = get_nc(S)
    in_maps = make_in_maps(hidden_states, W_qkv, b_qkv, W_proj, S)
    res = run_bass_kernel_spmd(nc, in_maps, list(range(NCORES))).results
    return gather(res, b_proj, S)


# revision 37
# speedup vs baseline: 1.0306x; 1.0306x over previous
"""Trainium2 Bass kernel for nn_CustomGPT2Attention (B=2, S=2048, D=1024, H=16).

Sharding: Megatron-style head-parallel over 8 cores (2 heads/core).
Each core computes QKV projection for its 2 heads, RoPE, causal
attention, and a row-parallel c_proj partial [D, T]; the host sums the
8 partials and adds b_proj.

Changes vs the fp32r baseline (295.7us -> ~219us):
  * all matmul operands bf16 (fp32 streams ~3x slower per column and
    disables FWL on LDWEIGHTS); PSUM accumulation stays fp32
  * softmax 1/den = exp(-ln den) on ACT, with the activation-table map
    patched so Ln and Exp share `natural_log_exp_and_others` (the stock
    placement pass picked two different sets and thrashed 17 x ~1.3us
    ACT_TABLE_LOADs per run)
  * the per-chunk normalize/proj-enqueue is deferred into the next
    chunk's attention body so the PE queue never parks on the
    rc-dependent broadcast matmul at a chunk boundary
  * chunk-interleaved program: attention (b, cc) starts as soon as its
    qkv chunks are done (first exp ~15us earlier); leftover qkv + proj
    units pace-fill attention gaps so the filler supply lasts to the end
  * diag-block exp issued as one 3D-AP ACTIVATE (fewer ACT fixed costs)
  * wqkv pre-swizzled on host to the SBUF layout (dense DMA, was a
    256B-element descriptor spray) and split per-ft; x chunk 0 is
    prefetched on the ACT HWDGE ring in parallel with the weights
  * outT + DMA in bf16 (halves output HBM traffic; host sums in f64)
  * proj PSUM->SBUF copies split across DVE and ACT to balance engines

Per-core data layout (features on partitions, "transposed"):
  xT      [D, T]    full hidden, transposed  (T = B*S tokens)
  qT/kT   [128, T]  partitions = (2 heads x 64 hd)
  scores  S^T tile [j=128, i<=512] so softmax'd probs feed the
          attn@V matmul directly as the moving operand; the two heads'
          score matmuls auto-row-tile (K=64 at partitions 0/64) and can
          run concurrently in the PE array
  V       [t, hd] via PE transpose, with a ones-column appended so the
          softmax denominator rides the attn@V matmul (M=65)
  1/den   exp(-ln den) on ACT (single shared table set), then a PE
          ones-matmul partition-broadcast
  out     partial^T [D, T], host sums across cores

The attention stream is ACT(exp)-paced; QKV and c_proj work is split
into small units and interleaved into the attention emission so the PE
FIFO stays dense (HAM stays at K=8/8) without starving the exp stream.
"""

import numpy as np
from collections import deque
from contextlib import ExitStack

import concourse.bass as bass
from concourse import bacc
import concourse.mybir as mybir
import concourse.tile as tile
from concourse.bass import ts, ds
from concourse.bass_utils import run_bass_kernel_spmd
from concourse.masks import make_identity, make_upper_triangular

F32 = mybir.dt.float32
EXP = mybir.ActivationFunctionType.Exp
LN = mybir.ActivationFunctionType.Ln

# Both Ln and Exp live in the `natural_log_exp_and_others` ACT table set,
# but the table-load placement pass resolves each function to the FIRST
# set containing it (exp -> exp_and_others, ln -> natural_log), which
# thrashes ACT_TABLE_LOADs (~2.7us each) every attention chunk. Restrict
# the advertised contents of those two sets (without disturbing set IDs)
# so both functions resolve to the shared set and one load covers all.
_ACT_TABLES_PATCHED = False


def _patch_act_tables():
    global _ACT_TABLES_PATCHED
    if _ACT_TABLES_PATCHED:
        return
    import concourse.hw_specs as hw_specs

    orig = hw_specs.get_activation_tables.__wrapped__

    @__import__("functools").cache
    def patched(module_arch):
        tables = {k: set(v) for k, v in orig(module_arch).items()}
        if "natural_log_exp_and_others" in tables:
            tables.get("exp_and_others", set()).discard(EXP)
            tables.get("natural_log", set()).discard(LN)
        return tables

    hw_specs.get_activation_tables = patched
    bacc.get_activation_tables = patched  # bacc holds a direct reference
    _ACT_TABLES_PATCHED = True

B, S, D = 2, 2048, 1024
H, HD = 16, 64
NCORES = 8
HPC = H // NCORES            # heads per core = 2
FL = HPC * HD                # local features = 128
THETA = 10000.0
TC = 512                     # token chunk (qkv / proj)
SC = 512                     # query chunk (attention)
JB = 128                     # key block
SCALE = 1.0 / 8.0            # 1/sqrt(HD)

MM_DT = mybir.dt.bfloat16    # matmul operand dtype


def build_nc(S_=S):
    _patch_act_tables()
    T = B * S_
    NCC = S_ // SC
    NTCB = S_ // TC
    NJT = T // JB
    NDT = D // 128

    nc = bacc.Bacc("TRN2", target_bir_lowering=False)
    xT = nc.declare_dram_parameter("xT", [D, T], MM_DT, isOutput=False)
    wqkv = nc.declare_dram_parameter("wqkv", [128, 3 * NDT * 128], MM_DT, isOutput=False)
    bqkv = nc.declare_dram_parameter("bqkv", [FL, 3], F32, isOutput=False)
    wproj = nc.declare_dram_parameter("wproj", [FL, D], MM_DT, isOutput=False)
    cos2 = nc.declare_dram_parameter("cos2", [FL, S_], MM_DT, isOutput=False)
    sin2s = nc.declare_dram_parameter("sin2s", [FL, S_], MM_DT, isOutput=False)
    outT = nc.declare_dram_parameter("outT", [D, T], MM_DT, isOutput=True)

    with tile.TileContext(nc) as tc:
        with ExitStack() as ctx:
            cpool = ctx.enter_context(tc.tile_pool(name="consts", bufs=1))
            big = ctx.enter_context(tc.tile_pool(name="big", bufs=1))
            xtp = ctx.enter_context(tc.tile_pool(name="xt", bufs=2))
            rpp = ctx.enter_context(tc.tile_pool(name="rope", bufs=2))
            ppp = ctx.enter_context(tc.tile_pool(name="pp", bufs=3))
            smp = ctx.enter_context(tc.tile_pool(name="small", bufs=2))
            stg = ctx.enter_context(tc.tile_pool(name="stg", bufs=3))
            mmps = ctx.enter_context(tc.tile_pool(name="mmps", bufs=2, space="PSUM"))
            scps = ctx.enter_context(tc.tile_pool(name="scps", bufs=2, space="PSUM"))
            ops = ctx.enter_context(tc.tile_pool(name="ops", bufs=1, space="PSUM"))

            # ---- weights first on the SP ring (QKV needs them first) ----
            # wqkv is pre-swizzled on the host to the exact SBUF layout and
            # split per-ft so the first matmul only waits on 1/3 of it
            wq_sb = [
                cpool.tile([128, NDT * 128], MM_DT, name=f"wq_sb{ft}")
                for ft in range(3)
            ]
            for ft in range(3):
                nc.sync.dma_start(wq_sb[ft][:], wqkv[:, ds(ft * NDT * 128, NDT * 128)])
            # ---- prefetch x chunk 0 on the ACT ring before the tables,
            # split in half so the first QKV matmuls start even earlier ----
            xT_r = xT.rearrange("(dk p) t -> p dk t", p=128)
            HK = NDT // 2
            xt0 = xtp.tile([128, NDT, TC], MM_DT, name="xt0")
            nc.scalar.dma_start(xt0[:, ds(0, HK), :], xT_r[:, ds(0, HK), ds(0, TC)])
            nc.scalar.dma_start(xt0[:, ds(HK, HK), :], xT_r[:, ds(HK, HK), ds(0, TC)])
            # ---- other constants on the ACT ring (parallel HWDGE ring) ----
            cos_sb = cpool.tile([128, S_], MM_DT)
            nc.scalar.dma_start(cos_sb[:], cos2[:, :])
            sin_sb = cpool.tile([128, S_], MM_DT)
            nc.scalar.dma_start(sin_sb[:], sin2s[:, :])
            bq_sb = cpool.tile([128, 3], F32)
            nc.scalar.dma_start(bq_sb[:], bqkv[:, :])
            wp_sb = cpool.tile([128, D], MM_DT)
            nc.scalar.dma_start(wp_sb[:], wproj[:, :])
            ident = cpool.tile([128, 128], MM_DT)
            make_identity(nc, ident[:])
            diagm = cpool.tile([128, 128], MM_DT)
            make_upper_triangular(nc, diagm[:], val=1.0, diag=True)
            ones64 = cpool.tile([1, 64], MM_DT)
            nc.vector.memset(ones64[:], 1.0)

            # ---- persistent activations ----
            q_sb = big.tile([128, T], MM_DT)
            k_sb = big.tile([128, T], MM_DT)
            vT_sb = big.tile([128, T], MM_DT)
            v_sb = big.tile([128, NJT * 130], MM_DT)  # [h0|1|h1|1] per block
            oT_sb = big.tile([128, T], MM_DT)
            nc.gpsimd.memset(v_sb[:], 1.0)

            # ------------------------------------------------------ units --
            def u_qkv_ft(b, cb, ft, xt):
                c = b * NTCB + cb
                t0 = c * TC
                if ft == 0 and c != 0:  # chunk 0 was prefetched above
                    nc.sync.dma_start(xt[:], xT_r[:, :, ds(t0, TC)])
                ps = mmps.tile([128, TC], F32, tag="mmps", name="ps")
                for dk in range(NDT):
                    nc.tensor.matmul(
                        ps[:],
                        wq_sb[ft][:, ts(dk, 128)],
                        xt[:, dk, :],
                        start=(dk == 0),
                        stop=(dk == NDT - 1),
                    )
                dst = (q_sb, k_sb, vT_sb)[ft]
                nc.vector.tensor_scalar_add(
                    dst[:, ds(t0, TC)], ps[:], bq_sb[:, ds(ft, 1)]
                )
                if ft >= 1:
                    # rope on q (ft==1) / k (ft==2) of this chunk
                    xsb = (q_sb, k_sb)[ft - 1]
                    s0 = t0 - b * S_
                    rot = rpp.tile([128, TC], MM_DT, tag="rot", name="rot")
                    for (po, pi) in ((0, 32), (32, 0), (64, 96), (96, 64)):
                        nc.gpsimd.dma_start(
                            rot[ds(po, 32), :], xsb[ds(pi, 32), ds(t0, TC)]
                        )
                    tmp = rpp.tile([128, TC], MM_DT, tag="tmp", name="tmp")
                    nc.vector.tensor_mul(
                        tmp[:], xsb[:, ds(t0, TC)], cos_sb[:, ds(s0, TC)]
                    )
                    nc.vector.tensor_mul(rot[:], rot[:], sin_sb[:, ds(s0, TC)])
                    nc.vector.tensor_add(xsb[:, ds(t0, TC)], tmp[:], rot[:])

            def u_vtrans(b, cb, jj):
                c = b * NTCB + cb
                jt = c * (TC // JB) + jj
                tp = mmps.tile([128, 128], MM_DT, tag="mmps", name="tp")
                nc.tensor.transpose(tp[:], vT_sb[:, ts(jt, JB)], ident[:])
                nc.vector.tensor_copy(
                    v_sb[:, ds(130 * jt, 130)].rearrange("p (g n) -> p g n", g=2)[
                        :, :, ds(0, 64)
                    ],
                    tp[:].rearrange("p (g n) -> p g n", g=2),
                )

            def u_proj(b, cc, dt):
                c = b * NTCB + cc
                pj = mmps.tile([128, TC], F32, tag="mmps", name="pj")
                nc.tensor.matmul(
                    pj[:], wp_sb[:, ts(dt, 128)], oT_sb[:, ts(c, TC)],
                    start=True, stop=True,
                )
                so = stg.tile([128, TC], MM_DT, tag="stg", name="so")
                if dt % 2 == 0:
                    nc.vector.tensor_copy(so[:], pj[:])
                else:
                    nc.scalar.copy(so[:], pj[:])
                nc.sync.dma_start(outT[ds(dt * 128, 128), ds(c * TC, TC)], so[:])

            UPC = 3 + TC // JB  # units per qkv chunk

            def qkv_units(b, cb):
                xt = xt0 if (b, cb) == (0, 0) else xtp.tile(
                    [128, NDT, TC], MM_DT, name="xt"
                )
                for ft in range(3):
                    yield (lambda b=b, cb=cb, ft=ft, xt=xt: u_qkv_ft(b, cb, ft, xt))
                for jj in range(TC // JB):
                    yield (lambda b=b, cb=cb, jj=jj: u_vtrans(b, cb, jj))

            fill_qkv = deque()
            fill_proj = deque()
            # pace filler consumption so the supply lasts through the final
            # attention chunk (an empty filler queue leaves the PE idling on
            # the exp stream, which re-throttles HAM)
            slots_left = [B * sum(4 * cc + 4 + 1 for cc in range(NCC))]

            def pop_filler():
                supply = len(fill_qkv) + len(fill_proj)
                k = min(2, max(1, -(-supply // max(slots_left[0], 1))))
                slots_left[0] -= 1
                for _ in range(k):
                    if fill_qkv:
                        fill_qkv.popleft()()
                    elif fill_proj:
                        fill_proj.popleft()()

            pending_finish = [None]

            def attn_finish(b, cc, oph2):
                # normalize: 1/d = exp(-ln d) on ACT, then PE broadcast.
                # Deferred into the NEXT chunk's attention body so the PE
                # queue never parks on the rc dependency at a boundary.
                i0 = b * S_ + cc * SC
                lnd = smp.tile([1, 2 * SC], F32, tag="lnd", name="lnd")
                nc.scalar.activation(lnd[:], oph2[ds(64, 1), :], LN)
                rc = smp.tile([1, 2 * SC], MM_DT, tag="rc", name="rc")
                nc.scalar.activation(rc[:], lnd[:], EXP, scale=-1.0)
                bcs = smp.tile([64, 2 * SC], MM_DT, tag="bcs", name="bcs")
                for h in range(2):
                    bcp = mmps.tile([64, SC], F32, tag="mmps", name="bcp")
                    nc.tensor.matmul(
                        bcp[:], ones64[:], rc[:, ds(SC * h, SC)],
                        start=True, stop=True,
                    )
                    nc.vector.tensor_copy(bcs[:, ds(SC * h, SC)], bcp[:])
                for h in range(2):
                    nc.vector.tensor_mul(
                        oT_sb[ds(64 * h, 64), ds(i0, SC)],
                        oph2[ds(0, 64), ds(SC * h, SC)],
                        bcs[:, ds(SC * h, SC)],
                    )
                for dt in range(NDT):
                    fill_proj.append(lambda b=b, cc=cc, dt=dt: u_proj(b, cc, dt))

            def emit_attn(b, cc):
                oph2 = ops.tile([65, 2 * SC], F32, tag="ops", name="oph2")
                nf = 4 * cc + 4

                def mk_scores(f):
                    ist = max(SC * cc, JB * f)
                    off = ist - SC * cc
                    N = SC - off
                    scp = scps.tile([128, 2 * SC], F32, tag="scps", name="scp")
                    for h in range(2):
                        nc.tensor.matmul(
                            scp[:, ds(SC * h + off, N)],
                            k_sb[ds(64 * h, 64), ds(b * S_ + JB * f, JB)],
                            q_sb[ds(64 * h, 64), ds(b * S_ + ist, N)],
                            start=True,
                            stop=True,
                        )
                    pp = ppp.tile([128, 2 * SC], MM_DT, tag="pp", name="pp")
                    if off == 0:
                        nc.scalar.activation(pp[:], scp[:], EXP, scale=SCALE)
                    else:
                        nc.scalar.activation(
                            pp[:].rearrange("p (g n) -> p g n", g=2)[
                                :, :, ds(off, N)
                            ],
                            scp[:].rearrange("p (g n) -> p g n", g=2)[
                                :, :, ds(off, N)
                            ],
                            EXP,
                            scale=SCALE,
                        )
                    if f >= 4 * cc:  # diagonal block: zero j > i
                        pp3 = pp[:].rearrange("p (g n) -> p g n", g=2)[
                            :, :, ds(off, JB)
                        ]
                        nc.vector.tensor_mul(
                            pp3, pp3, diagm[:].unsqueeze(1).to_broadcast((128, 2, JB))
                        )
                    return pp, off, N

                def mk_attnv(f, pp, off, N):
                    jt = b * (S_ // JB) + f
                    for h in range(2):
                        nc.tensor.matmul(
                            oph2[:, ds(SC * h + off, N)],
                            v_sb[:, ds(130 * jt + 65 * h, 65)],
                            pp[:, ds(SC * h + off, N)],
                            start=(f == 0),
                            stop=(f == nf - 1),
                        )

                # software-pipelined: scores run one f ahead of attn@V so the
                # PE FIFO never parks on an exp-dependent matmul; the prior
                # chunk's finish work slots in behind the first scores
                prev = mk_scores(0)
                if pending_finish[0] is not None:
                    # prior chunk's finish slots in behind the first scores
                    pending_finish[0]()
                    pending_finish[0] = None
                for f in range(1, nf):
                    cur = mk_scores(f)
                    mk_attnv(f - 1, *prev)
                    prev = cur
                    pop_filler()
                mk_attnv(nf - 1, *prev)
                pop_filler()
                pop_filler()
                pending_finish[0] = lambda: attn_finish(b, cc, oph2)

            # ---------------------------------------------------- program --
            # chunk-interleaved: attention for (b, cc) starts as soon as the
            # qkv chunks it reads are done; remaining qkv/proj work fills
            # attention gaps so the PE stays dense (HAM stays warm)
            total_qkv = (2 * NTCB - 1) * UPC
            for u in qkv_units(0, 0):
                u()
            for bb in range(B):
                for cb in range(NTCB):
                    if (bb, cb) != (0, 0):
                        fill_qkv.extend(qkv_units(bb, cb))
            assert len(fill_qkv) == total_qkv

            def chunks_ready():
                return 1 + (total_qkv - len(fill_qkv)) // UPC

            for bb in range(B):
                for cc in range(NCC):
                    g = bb * NCC + cc  # needs qkv chunks 0..g
                    while fill_qkv and chunks_ready() < g + 1:
                        fill_qkv.popleft()()
                    emit_attn(bb, cc)
            if pending_finish[0] is not None:
                pending_finish[0]()
                pending_finish[0] = None
            while fill_qkv:
                fill_qkv.popleft()()
            while fill_proj:
                fill_proj.popleft()()

    nc.finalize()
    return nc


# ---------------------------------------------------------------------------
# host side
# ---------------------------------------------------------------------------

def rope_tables(S_=S):
    hd_half = HD // 2
    inv = (
        np.float32(1.0)
        / np.float32(THETA) ** (np.arange(0, HD, 2, dtype=np.float32) / np.float32(HD))
    ).astype(np.float32)
    t = np.arange(S_, dtype=np.float32)
    freqs = np.outer(t, inv).astype(np.float32)
    emb = np.concatenate([freqs, freqs], axis=1)
    cos = np.cos(emb).astype(np.float32)
    sin = np.sin(emb).astype(np.float32)
    sign = np.where(np.arange(HD) < hd_half, np.float32(-1.0), np.float32(1.0))
    cos2 = np.tile(cos.T, (HPC, 1)).astype(np.float32)
    sin2s = np.tile((sin * sign[None, :]).T, (HPC, 1)).astype(np.float32)
    return np.ascontiguousarray(cos2), np.ascontiguousarray(sin2s)


def make_in_maps(hidden_states, W_qkv, b_qkv, W_proj, S_=S):
    T = B * S_
    mmnp = mybir.dt.np(MM_DT)
    x = np.asarray(hidden_states, dtype=np.float32).reshape(T, D)
    xT = np.ascontiguousarray(x.T).astype(mmnp)
    cos2, sin2s = rope_tables(S_)
    cos2 = cos2.astype(mmnp)
    sin2s = sin2s.astype(mmnp)
    maps = []
    NDT = D // 128
    for i in range(NCORES):
        cs = slice(FL * i, FL * (i + 1))
        # pre-swizzle to the SBUF layout [p, (ft dk c)] so the device DMA
        # is one dense per-partition transfer
        w3 = np.stack([W_qkv[:, k * D:][:, cs] for k in range(3)], axis=0)
        w3 = w3.reshape(3, NDT, 128, FL).transpose(2, 0, 1, 3)
        wq = np.ascontiguousarray(w3.reshape(128, 3 * NDT * FL)).astype(mmnp)
        bq = np.ascontiguousarray(
            np.stack([b_qkv[k * D:][cs] for k in range(3)], axis=1)
        ).astype(np.float32)
        wp = np.ascontiguousarray(W_proj[cs, :]).astype(mmnp)
        maps.append(dict(xT=xT, wqkv=wq, bqkv=bq, wproj=wp, cos2=cos2, sin2s=sin2s))
    return maps


_NC_CACHE = {}


def get_nc(S_=S):
    if S_ not in _NC_CACHE:
        _NC_CACHE[S_] = build_nc(S_)
    return _NC_CACHE[S_]


def gather(results, b_proj, S_=S):
    acc = np.zeros((D, B * S_), dtype=np.float64)
    for r in results:
        acc += np.asarray(r["outT"], dtype=np.float64)
    out = acc.T + np.asarray(b_proj, dtype=np.float64)[None, :]
    return out.astype(np.float32).reshape(B, S_, D)


def kernel(hidden_states, W_qkv, b_qkv, W_proj, b_proj):
    nc = get_nc(S)
    in_maps = make_in_maps(hidden_states, W_qkv, b_qkv, W_proj, S)
    res = run_bass_kernel_spmd(nc, in_maps, list(range(NCORES))).results
    return gather(res, b_proj, S)


# revision 39
# speedup vs baseline: 1.0516x; 1.0203x over previous
"""Trainium2 Bass kernel for nn_CustomGPT2Attention (B=2, S=2048, D=1024, H=16).

Sharding: Megatron-style head-parallel over 8 cores (2 heads/core).
Each core computes QKV projection for its 2 heads, RoPE, causal
attention, and a row-parallel c_proj partial [D, T]; the host sums the
8 partials and adds b_proj.

Changes vs the fp32r baseline (295.7us -> ~219us):
  * all matmul operands bf16 (fp32 streams ~3x slower per column and
    disables FWL on LDWEIGHTS); PSUM accumulation stays fp32
  * softmax 1/den = exp(-ln den) on ACT, with the activation-table map
    patched so Ln and Exp share `natural_log_exp_and_others` (the stock
    placement pass picked two different sets and thrashed 17 x ~1.3us
    ACT_TABLE_LOADs per run)
  * the per-chunk normalize/proj-enqueue is deferred into the next
    chunk's attention body so the PE queue never parks on the
    rc-dependent broadcast matmul at a chunk boundary
  * chunk-interleaved program: attention (b, cc) starts as soon as its
    qkv chunks are done (first exp ~15us earlier); leftover qkv + proj
    units pace-fill attention gaps so the filler supply lasts to the end
  * diag-block exp issued as one 3D-AP ACTIVATE (fewer ACT fixed costs)
  * wqkv pre-swizzled on host to the SBUF layout (dense DMA, was a
    256B-element descriptor spray) and split per-ft; x chunk 0 is
    prefetched on the ACT HWDGE ring in parallel with the weights
  * outT + DMA in bf16 (halves output HBM traffic; host sums in f64)
  * proj PSUM->SBUF copies split across DVE and ACT to balance engines

Per-core data layout (features on partitions, "transposed"):
  xT      [D, T]    full hidden, transposed  (T = B*S tokens)
  qT/kT   [128, T]  partitions = (2 heads x 64 hd)
  scores  S^T tile [j=128, i<=512] so softmax'd probs feed the
          attn@V matmul directly as the moving operand; the two heads'
          score matmuls auto-row-tile (K=64 at partitions 0/64) and can
          run concurrently in the PE array
  V       [t, hd] via PE transpose, with a ones-column appended so the
          softmax denominator rides the attn@V matmul (M=65)
  1/den   exp(-ln den) on ACT (single shared table set), then a PE
          ones-matmul partition-broadcast
  out     partial^T [D, T], host sums across cores

The attention stream is ACT(exp)-paced; QKV and c_proj work is split
into small units and interleaved into the attention emission so the PE
FIFO stays dense (HAM stays at K=8/8) without starving the exp stream.
"""

import numpy as np
from collections import deque
from contextlib import ExitStack

import concourse.bass as bass
from concourse import bacc
import concourse.mybir as mybir
import concourse.tile as tile
from concourse.bass import ts, ds
from concourse.bass_utils import run_bass_kernel_spmd
from concourse.masks import make_identity, make_upper_triangular

F32 = mybir.dt.float32
EXP = mybir.ActivationFunctionType.Exp
LN = mybir.ActivationFunctionType.Ln

# Both Ln and Exp live in the `natural_log_exp_and_others` ACT table set,
# but the table-load placement pass resolves each function to the FIRST
# set containing it (exp -> exp_and_others, ln -> natural_log), which
# thrashes ACT_TABLE_LOADs (~2.7us each) every attention chunk. Restrict
# the advertised contents of those two sets (without disturbing set IDs)
# so both functions resolve to the shared set and one load covers all.
_ACT_TABLES_PATCHED = False


def _patch_act_tables():
    global _ACT_TABLES_PATCHED
    if _ACT_TABLES_PATCHED:
        return
    import concourse.hw_specs as hw_specs

    orig = hw_specs.get_activation_tables.__wrapped__

    @__import__("functools").cache
    def patched(module_arch):
        tables = {k: set(v) for k, v in orig(module_arch).items()}
        if "natural_log_exp_and_others" in tables:
            tables.get("exp_and_others", set()).discard(EXP)
            tables.get("natural_log", set()).discard(LN)
        return tables

    hw_specs.get_activation_tables = patched
    bacc.get_activation_tables = patched  # bacc holds a direct reference
    _ACT_TABLES_PATCHED = True

B, S, D = 2, 2048, 1024
H, HD = 16, 64
NCORES = 8
HPC = H // NCORES            # heads per core = 2
FL = HPC * HD                # local features = 128
THETA = 10000.0
TC = 512                     # token chunk (qkv / proj)
SC = 512                     # query chunk (attention)
JB = 128                     # key block
SCALE = 1.0 / 8.0            # 1/sqrt(HD)

MM_DT = mybir.dt.bfloat16    # matmul operand dtype


def build_nc(S_=S):
    _patch_act_tables()
    T = B * S_
    NCC = S_ // SC
    NTCB = S_ // TC
    NJT = T // JB
    NDT = D // 128

    nc = bacc.Bacc("TRN2", target_bir_lowering=False)
    xT = nc.declare_dram_parameter("xT", [D, T], MM_DT, isOutput=False)
    wqkv = nc.declare_dram_parameter("wqkv", [128, 3 * NDT * 128], MM_DT, isOutput=False)
    bqkv = nc.declare_dram_parameter("bqkv", [FL, 3], F32, isOutput=False)
    wproj = nc.declare_dram_parameter("wproj", [FL, D], MM_DT, isOutput=False)
    cos2 = nc.declare_dram_parameter("cos2", [FL, S_], MM_DT, isOutput=False)
    sin2s = nc.declare_dram_parameter("sin2s", [FL, S_], MM_DT, isOutput=False)
    outT = nc.declare_dram_parameter("outT", [D, T], MM_DT, isOutput=True)

    with tile.TileContext(nc) as tc:
        with ExitStack() as ctx:
            cpool = ctx.enter_context(tc.tile_pool(name="consts", bufs=1))
            big = ctx.enter_context(tc.tile_pool(name="big", bufs=1))
            xtp = ctx.enter_context(tc.tile_pool(name="xt", bufs=2))
            rpp = ctx.enter_context(tc.tile_pool(name="rope", bufs=2))
            ppp = ctx.enter_context(tc.tile_pool(name="pp", bufs=4))
            smp = ctx.enter_context(tc.tile_pool(name="small", bufs=2))
            stg = ctx.enter_context(tc.tile_pool(name="stg", bufs=3))
            mmps = ctx.enter_context(tc.tile_pool(name="mmps", bufs=2, space="PSUM"))
            scps = ctx.enter_context(tc.tile_pool(name="scps", bufs=2, space="PSUM"))
            ops = ctx.enter_context(tc.tile_pool(name="ops", bufs=1, space="PSUM"))

            # ---- weights first on the SP ring (QKV needs them first) ----
            # wqkv is pre-swizzled on the host to the exact SBUF layout and
            # split per-ft so the first matmul only waits on 1/3 of it
            wq_sb = [
                cpool.tile([128, NDT * 128], MM_DT, name=f"wq_sb{ft}")
                for ft in range(3)
            ]
            for ft in range(3):
                nc.sync.dma_start(wq_sb[ft][:], wqkv[:, ds(ft * NDT * 128, NDT * 128)])
            # ---- prefetch x chunk 0 on the ACT ring before the tables,
            # split in half so the first QKV matmuls start even earlier ----
            xT_r = xT.rearrange("(dk p) t -> p dk t", p=128)
            HK = NDT // 2
            xt0 = xtp.tile([128, NDT, TC], MM_DT, name="xt0")
            nc.scalar.dma_start(xt0[:, ds(0, HK), :], xT_r[:, ds(0, HK), ds(0, TC)])
            nc.scalar.dma_start(xt0[:, ds(HK, HK), :], xT_r[:, ds(HK, HK), ds(0, TC)])
            # ---- other constants on the ACT ring (parallel HWDGE ring) ----
            cos_sb = cpool.tile([128, S_], MM_DT)
            nc.scalar.dma_start(cos_sb[:], cos2[:, :])
            sin_sb = cpool.tile([128, S_], MM_DT)
            nc.scalar.dma_start(sin_sb[:], sin2s[:, :])
            bq_sb = cpool.tile([128, 3], F32)
            nc.scalar.dma_start(bq_sb[:], bqkv[:, :])
            wp_sb = cpool.tile([128, D], MM_DT)
            nc.scalar.dma_start(wp_sb[:], wproj[:, :])
            ident = cpool.tile([128, 128], MM_DT)
            make_identity(nc, ident[:])
            diagm = cpool.tile([128, 128], MM_DT)
            make_upper_triangular(nc, diagm[:], val=1.0, diag=True)
            ones64 = cpool.tile([1, 64], MM_DT)
            nc.vector.memset(ones64[:], 1.0)

            # ---- persistent activations ----
            q_sb = big.tile([128, T], MM_DT)
            k_sb = big.tile([128, T], MM_DT)
            vT_sb = big.tile([128, T], MM_DT)
            v_sb = big.tile([128, NJT * 130], MM_DT)  # [h0|1|h1|1] per block
            oT_sb = big.tile([128, T], MM_DT)
            nc.gpsimd.memset(v_sb[:], 1.0)

            # ------------------------------------------------------ units --
            def u_qkv_ft(b, cb, ft, xt):
                c = b * NTCB + cb
                t0 = c * TC
                if ft == 0 and c != 0:  # chunk 0 was prefetched above
                    nc.sync.dma_start(xt[:], xT_r[:, :, ds(t0, TC)])
                ps = mmps.tile([128, TC], F32, tag="mmps", name="ps")
                for dk in range(NDT):
                    nc.tensor.matmul(
                        ps[:],
                        wq_sb[ft][:, ts(dk, 128)],
                        xt[:, dk, :],
                        start=(dk == 0),
                        stop=(dk == NDT - 1),
                    )
                dst = (q_sb, k_sb, vT_sb)[ft]
                nc.vector.tensor_scalar_add(
                    dst[:, ds(t0, TC)], ps[:], bq_sb[:, ds(ft, 1)]
                )
                if ft >= 1:
                    # rope on q (ft==1) / k (ft==2) of this chunk
                    xsb = (q_sb, k_sb)[ft - 1]
                    s0 = t0 - b * S_
                    rot = rpp.tile([128, TC], MM_DT, tag="rot", name="rot")
                    for (po, pi) in ((0, 32), (32, 0), (64, 96), (96, 64)):
                        nc.gpsimd.dma_start(
                            rot[ds(po, 32), :], xsb[ds(pi, 32), ds(t0, TC)]
                        )
                    tmp = rpp.tile([128, TC], MM_DT, tag="tmp", name="tmp")
                    nc.vector.tensor_mul(
                        tmp[:], xsb[:, ds(t0, TC)], cos_sb[:, ds(s0, TC)]
                    )
                    nc.vector.tensor_mul(rot[:], rot[:], sin_sb[:, ds(s0, TC)])
                    nc.vector.tensor_add(xsb[:, ds(t0, TC)], tmp[:], rot[:])

            def u_vtrans(b, cb, jj):
                c = b * NTCB + cb
                jt = c * (TC // JB) + jj
                tp = mmps.tile([128, 128], MM_DT, tag="mmps", name="tp")
                nc.tensor.transpose(tp[:], vT_sb[:, ts(jt, JB)], ident[:])
                nc.vector.tensor_copy(
                    v_sb[:, ds(130 * jt, 130)].rearrange("p (g n) -> p g n", g=2)[
                        :, :, ds(0, 64)
                    ],
                    tp[:].rearrange("p (g n) -> p g n", g=2),
                )

            def u_proj(b, cc, dt):
                c = b * NTCB + cc
                pj = mmps.tile([128, TC], F32, tag="mmps", name="pj")
                nc.tensor.matmul(
                    pj[:], wp_sb[:, ts(dt, 128)], oT_sb[:, ts(c, TC)],
                    start=True, stop=True,
                )
                so = stg.tile([128, TC], MM_DT, tag="stg", name="so")
                if dt % 2 == 0:
                    nc.vector.tensor_copy(so[:], pj[:])
                else:
                    nc.scalar.copy(so[:], pj[:])
                nc.sync.dma_start(outT[ds(dt * 128, 128), ds(c * TC, TC)], so[:])

            UPC = 3 + TC // JB  # units per qkv chunk

            def qkv_units(b, cb):
                xt = xt0 if (b, cb) == (0, 0) else xtp.tile(
                    [128, NDT, TC], MM_DT, name="xt"
                )
                for ft in range(3):
                    yield (lambda b=b, cb=cb, ft=ft, xt=xt: u_qkv_ft(b, cb, ft, xt))
                for jj in range(TC // JB):
                    yield (lambda b=b, cb=cb, jj=jj: u_vtrans(b, cb, jj))

            fill_qkv = deque()
            fill_proj = deque()
            # pace filler consumption so the supply lasts through the final
            # attention chunk (an empty filler queue leaves the PE idling on
            # the exp stream, which re-throttles HAM)
            slots_left = [B * sum(4 * cc + 4 + 1 for cc in range(NCC))]

            def pop_filler():
                supply = len(fill_qkv) + len(fill_proj)
                k = min(2, max(1, -(-supply // max(slots_left[0], 1))))
                slots_left[0] -= 1
                for _ in range(k):
                    if fill_qkv:
                        fill_qkv.popleft()()
                    elif fill_proj:
                        fill_proj.popleft()()

            pending_finish = [None]

            def attn_finish(b, cc, oph2):
                # normalize: 1/d = exp(-ln d) on ACT, then PE broadcast.
                # Deferred into the NEXT chunk's attention body so the PE
                # queue never parks on the rc dependency at a boundary.
                i0 = b * S_ + cc * SC
                lnd = smp.tile([1, 2 * SC], F32, tag="lnd", name="lnd")
                nc.scalar.activation(lnd[:], oph2[ds(64, 1), :], LN)
                rc = smp.tile([1, 2 * SC], MM_DT, tag="rc", name="rc")
                nc.scalar.activation(rc[:], lnd[:], EXP, scale=-1.0)
                bcs = smp.tile([64, 2 * SC], MM_DT, tag="bcs", name="bcs")
                for h in range(2):
                    bcp = mmps.tile([64, SC], F32, tag="mmps", name="bcp")
                    nc.tensor.matmul(
                        bcp[:], ones64[:], rc[:, ds(SC * h, SC)],
                        start=True, stop=True,
                    )
                    nc.vector.tensor_copy(bcs[:, ds(SC * h, SC)], bcp[:])
                for h in range(2):
                    nc.vector.tensor_mul(
                        oT_sb[ds(64 * h, 64), ds(i0, SC)],
                        oph2[ds(0, 64), ds(SC * h, SC)],
                        bcs[:, ds(SC * h, SC)],
                    )
                for dt in range(NDT):
                    fill_proj.append(lambda b=b, cc=cc, dt=dt: u_proj(b, cc, dt))

            def emit_attn(b, cc):
                oph2 = ops.tile([65, 2 * SC], F32, tag="ops", name="oph2")
                nf = 4 * cc + 4

                def mk_scores(f):
                    ist = max(SC * cc, JB * f)
                    off = ist - SC * cc
                    N = SC - off
                    scp = scps.tile([128, 2 * SC], F32, tag="scps", name="scp")
                    for h in range(2):
                        nc.tensor.matmul(
                            scp[:, ds(SC * h + off, N)],
                            k_sb[ds(64 * h, 64), ds(b * S_ + JB * f, JB)],
                            q_sb[ds(64 * h, 64), ds(b * S_ + ist, N)],
                            start=True,
                            stop=True,
                        )
                    pp = ppp.tile([128, 2 * SC], MM_DT, tag="pp", name="pp")
                    if off == 0:
                        nc.scalar.activation(pp[:], scp[:], EXP, scale=SCALE)
                    else:
                        nc.scalar.activation(
                            pp[:].rearrange("p (g n) -> p g n", g=2)[
                                :, :, ds(off, N)
                            ],
                            scp[:].rearrange("p (g n) -> p g n", g=2)[
                                :, :, ds(off, N)
                            ],
                            EXP,
                            scale=SCALE,
                        )
                    if f >= 4 * cc:  # diagonal block: zero j > i
                        pp3 = pp[:].rearrange("p (g n) -> p g n", g=2)[
                            :, :, ds(off, JB)
                        ]
                        nc.vector.tensor_mul(
                            pp3, pp3, diagm[:].unsqueeze(1).to_broadcast((128, 2, JB))
                        )
                    return pp, off, N

                def mk_attnv(f, pp, off, N):
                    jt = b * (S_ // JB) + f
                    for h in range(2):
                        nc.tensor.matmul(
                            oph2[:, ds(SC * h + off, N)],
                            v_sb[:, ds(130 * jt + 65 * h, 65)],
                            pp[:, ds(SC * h + off, N)],
                            start=(f == 0),
                            stop=(f == nf - 1),
                        )

                # software-pipelined: scores run one f ahead of attn@V so the
                # PE FIFO never parks on an exp-dependent matmul; the prior
                # chunk's finish work slots in behind the first scores
                prev = mk_scores(0)
                if pending_finish[0] is not None:
                    # prior chunk's finish slots in behind the first scores
                    pending_finish[0]()
                    pending_finish[0] = None
                for f in range(1, nf):
                    cur = mk_scores(f)
                    mk_attnv(f - 1, *prev)
                    prev = cur
                    pop_filler()
                mk_attnv(nf - 1, *prev)
                pop_filler()
                pop_filler()
                pending_finish[0] = lambda: attn_finish(b, cc, oph2)

            # ---------------------------------------------------- program --
            # chunk-interleaved: attention for (b, cc) starts as soon as the
            # qkv chunks it reads are done; remaining qkv/proj work fills
            # attention gaps so the PE stays dense (HAM stays warm)
            total_qkv = (2 * NTCB - 1) * UPC
            for u in qkv_units(0, 0):
                u()
            for bb in range(B):
                for cb in range(NTCB):
                    if (bb, cb) != (0, 0):
                        fill_qkv.extend(qkv_units(bb, cb))
            assert len(fill_qkv) == total_qkv

            def chunks_ready():
                return 1 + (total_qkv - len(fill_qkv)) // UPC

            # batch 1's chunks run largest-first: the nf=16 chunk then sits
            # mid-stream with a deep filler backlog, and the kernel ends on
            # the nf=4 chunk instead of a starved ACT-paced stretch
            for bb in range(B):
                cc_order = range(NCC) if bb == 0 else range(NCC - 1, -1, -1)
                for cc in cc_order:
                    g = bb * NCC + cc  # needs qkv chunks 0..g
                    while fill_qkv and chunks_ready() < g + 1:
                        fill_qkv.popleft()()
                    emit_attn(bb, cc)
            if pending_finish[0] is not None:
                pending_finish[0]()
                pending_finish[0] = None
            while fill_qkv:
                fill_qkv.popleft()()
            while fill_proj:
                fill_proj.popleft()()

    nc.finalize()
    return nc


# ---------------------------------------------------------------------------
# host side
# ---------------------------------------------------------------------------

def rope_tables(S_=S):
    hd_half = HD // 2
    inv = (
        np.float32(1.0)
        / np.float32(THETA) ** (np.arange(0, HD, 2, dtype=np.float32) / np.float32(HD))
    ).astype(np.float32)
    t = np.arange(S_, dtype=np.float32)
    freqs = np.outer(t, inv).astype(np.float32)
    emb = np.concatenate([freqs, freqs], axis=1)
    cos = np.cos(emb).astype(np.float32)
    sin = np.sin(emb).astype(np.float32)
    sign = np.where(np.arange(HD) < hd_half, np.float32(-1.0), np.float32(1.0))
    cos2 = np.tile(cos.T, (HPC, 1)).astype(np.float32)
    sin2s = np.tile((sin * sign[None, :]).T, (HPC, 1)).astype(np.float32)
    return np.ascontiguousarray(cos2), np.ascontiguousarray(sin2s)


def make_in_maps(hidden_states, W_qkv, b_qkv, W_proj, S_=S):
    T = B * S_
    mmnp = mybir.dt.np(MM_DT)
    x = np.asarray(hidden_states, dtype=np.float32).reshape(T, D)
    xT = np.ascontiguousarray(x.T).astype(mmnp)
    cos2, sin2s = rope_tables(S_)
    cos2 = cos2.astype(mmnp)
    sin2s = sin2s.astype(mmnp)
    maps = []
    NDT = D // 128
    for i in range(NCORES):
        cs = slice(FL * i, FL * (i + 1))
        # pre-swizzle to the SBUF layout [p, (ft dk c)] so the device DMA
        # is one dense per-partition transfer
        w3 = np.stack([W_qkv[:, k * D:][:, cs] for k in range(3)], axis=0)
        w3 = w3.reshape(3, NDT, 128, FL).transpose(2, 0, 1, 3)
        wq = np.ascontiguousarray(w3.reshape(128, 3 * NDT * FL)).astype(mmnp)
        bq = np.ascontiguousarray(
            np.stack([b_qkv[k * D:][cs] for k in range(3)], axis=1)
        ).astype(np.float32)
        wp = np.ascontiguousarray(W_proj[cs, :]).astype(mmnp)
        maps.append(dict(xT=xT, wqkv=wq, bqkv=bq, wproj=wp, cos2=cos2, sin2s=sin2s))
    return maps


_NC_CACHE = {}


def get_nc(S_=S):
    if S_ not in _NC_CACHE:
        _NC_CACHE[S_] = build_nc(S_)
    return _NC_CACHE[S_]


def gather(results, b_proj, S_=S):
    acc = np.zeros((D, B * S_), dtype=np.float64)
    for r in results:
        acc += np.asarray(r["outT"], dtype=np.float64)
    out = acc.T + np.asarray(b_proj, dtype=np.float64)[None, :]
    return out.astype(np.float32).reshape(B, S_, D)


def kernel(hidden_states, W_qkv, b_qkv, W_proj, b_proj):
    nc = get_nc(S)
    in_maps = make_in_maps(hidden_states, W_qkv, b_qkv, W_proj, S)
    res = run_bass_kernel_spmd(nc, in_maps, list(range(NCORES))).results
    return gather(res, b_proj, S)
